# revision 1
# baseline (speedup 1.0000x reference)
"""Bass/Trainium2 kernel for nn_EquivSetGNN3 (gnn_message_passing).

Math (reference): x = relu(x@W_in+b_in); x0 = x
  2 layers of: Xe = segsum_E((x@W1+b1)[V]); Xev = cat(x[V], Xe[E])@W2+b2
               Xv = segsum_V(Xev); x = relu((0.5*Xv + 0.5*x0)@W3 + b3)

Algebraic restructuring (avoids all [nnz, C] feature materialization):
  Xe = (segsum_E x[V]) @ W1 + deg_E (x) b1
  Xv = deg_V (*) (x @ W2a) + (segsum_V Xe[E]) @ W2b + deg_V (x) b2
where W2a = W2[:C], W2b = W2[C:].

Segment sums are done on-device as: dma_gather of 512B rows + one-hot
matmuls on the TensorEngine (128 incidences -> <=128 segments per chunk,
accumulated in PSUM over a "superchunk" of 128 segments).

Sharding: nodes and edges split 8 ways (graph parallel); x and Xe are
AllGathered between phases; weights replicated.
"""
import numpy as np

import concourse.bacc as bacc
import concourse.mybir as mybir
import concourse.tile as tile
from concourse.bass_utils import run_bass_kernel_spmd

f32 = mybir.dt.float32
f16 = mybir.dt.float16
i16 = mybir.dt.int16
SEG_DT = f16  # dtype for gathered-feature storage + segment matmuls

N = 50000
M = 25000
NNZ = 800000
C = 128
R = 8
NO = N // R          # 6250 nodes per core
EO = M // R          # 3125 edges per core
SPLIT = 32768        # int16 positive range limit for gather indices
SCA = (EO + 127) // 128   # 25 edge superchunks per core
SCB = (NO + 127) // 128   # 49 node superchunks per core
N_LAYERS = 2
ALPHA = 0.5

_cache = {}
DEBUG_STAGE = 99  # build stages up to this (bisection aid)
X_FULL_ADDR_SPACE = "Local"  # "Shared" is faster for collectives but untested w/ gather
PA_LEVEL = 9  # phase-A internal bisection: 1=gathers 2=+seg matmuls 3=+w1/bias 4=+transpose/out


def set_problem(n, m):
    """Override problem sizes (testing only)."""
    global N, M, NO, EO, SCA, SCB
    N, M = n, m
    NO, EO = N // R, M // R
    SCA = (EO + 127) // 128
    SCB = (NO + 127) // 128
    _cache.clear()


def _wrap_idx(flat):
    """[L] int -> [128, L//16] int16 (idx i at partition i%16, col i//16;
    replicated 8x across partition groups for the 8 gpsimd cores)."""
    w = flat.reshape(-1, 16).T.astype(np.int16)
    return np.ascontiguousarray(np.tile(w, (8, 1)))


def _wrap_rel(flat):
    """[L] int -> [128, L//128] f32 (value i at partition i%128, col i//128)."""
    return np.ascontiguousarray(flat.reshape(-1, 128).T.astype(np.float32))


def _make_P(flat):
    """[L] rel ids -> one-hot P [128, L] fp16: P[p, c*128+j] = (rel[c*128+p]==j).
    Padded entries (rel == -1) give all-zero rows."""
    rel = flat.reshape(-1, 128)          # [nch, 128] (chunk, partition)
    oh = rel[:, :, None] == np.arange(128)[None, None, :]  # [nch, p, j]
    return np.ascontiguousarray(
        oh.transpose(1, 0, 2).reshape(128, -1).astype(np.float16))


def _pad_to(arr, n, val):
    out = np.full(n, val, dtype=arr.dtype)
    out[: len(arr)] = arr
    return out


def _prepare(V, E):
    """Host-side preprocessing: sorted/sharded/padded gather index+rel arrays.

    Returns (meta, per_core) where meta has the (core-independent) chunk
    structure and per_core the index/rel arrays per core.
    """
    # ---- phase A: incidences sorted by E (edge-major) ----
    oA = np.argsort(E, kind="stable")
    Va, Ea = V[oA], E[oA]
    # per (core, superchunk) edge windows
    e0 = np.arange(R)[:, None] * EO + np.arange(SCA)[None, :] * 128  # [R,SCA]
    e1 = np.minimum(e0 + 128, (np.arange(R)[:, None] + 1) * EO)
    lo = np.searchsorted(Ea, e0.ravel()).reshape(R, SCA)
    hi = np.searchsorted(Ea, e1.ravel()).reshape(R, SCA)

    cntA = np.zeros((R, SCA), np.int64)
    cntB = np.zeros((R, SCA), np.int64)
    for r in range(R):
        for s in range(SCA):
            seg = Va[lo[r, s] : hi[r, s]]
            nb = int((seg >= SPLIT).sum())
            cntB[r, s] = nb
            cntA[r, s] = len(seg) - nb
    nchA = (-(-cntA.max(0) // 128)).astype(np.int64)  # [SCA] chunks, cross-core max
    nchB = (-(-cntB.max(0) // 128)).astype(np.int64)

    # ---- phase B: incidences sorted by V (node-major) ----
    oB = np.argsort(V, kind="stable")
    Vb, Eb = V[oB], E[oB]
    v0 = np.arange(R)[:, None] * NO + np.arange(SCB)[None, :] * 128
    v1 = np.minimum(v0 + 128, (np.arange(R)[:, None] + 1) * NO)
    lo2 = np.searchsorted(Vb, v0.ravel()).reshape(R, SCB)
    hi2 = np.searchsorted(Vb, v1.ravel()).reshape(R, SCB)
    cnt2 = hi2 - lo2
    nch2 = (-(-cnt2.max(0) // 128)).astype(np.int64)  # [SCB]

    meta = {
        "nchA": nchA.tolist(),
        "nchB": nchB.tolist(),
        "nch2": nch2.tolist(),
    }

    degE = np.bincount(E, minlength=M).astype(np.float32)
    degV = np.bincount(V, minlength=N).astype(np.float32)

    per_core = []
    for r in range(R):
        idxA_parts, relA_parts = [], []
        for s in range(SCA):
            seg_v = Va[lo[r, s] : hi[r, s]]
            seg_e = Ea[lo[r, s] : hi[r, s]] - e0[r, s]
            mB = seg_v >= SPLIT
            # bucket A (V < SPLIT)
            la = int(nchA[s]) * 128
            idxA_parts.append(_pad_to(seg_v[~mB], la, 0))
            relA_parts.append(_pad_to(seg_e[~mB], la, -1))
            # bucket B (V >= SPLIT, rebased)
            lb = int(nchB[s]) * 128
            idxA_parts.append(_pad_to(seg_v[mB] - SPLIT, lb, 0))
            relA_parts.append(_pad_to(seg_e[mB], lb, -1))
        idxA = np.concatenate(idxA_parts)
        relA = np.concatenate(relA_parts)

        idxB_parts, relB_parts = [], []
        for s in range(SCB):
            seg_e = Eb[lo2[r, s] : hi2[r, s]]
            seg_v = Vb[lo2[r, s] : hi2[r, s]] - v0[r, s]
            lb = int(nch2[s]) * 128
            idxB_parts.append(_pad_to(seg_e, lb, 0))
            relB_parts.append(_pad_to(seg_v, lb, -1))
        idxB = np.concatenate(idxB_parts)
        relB = np.concatenate(relB_parts)

        per_core.append(
            {
                "idxA": _wrap_idx(idxA),
                "PA": _make_P(relA),
                "idxB": _wrap_idx(idxB),
                "PB": _make_P(relB),
                "degE": degE[r * EO : (r + 1) * EO],
                "degV": degV[r * NO : (r + 1) * NO],
            }
        )
    return meta, per_core


def _build(meta):
    nchA = meta["nchA"]
    nchB = meta["nchB"]
    nch2 = meta["nch2"]
    LA = sum(a + b for a, b in zip(nchA, nchB)) * 128
    LB = sum(nch2) * 128

    nc = bacc.Bacc("TRN2", target_bir_lowering=False, debug=False, num_devices=R,
                   num_swdge_queues=4)

    # ---- kernel I/O ----
    xsh = nc.declare_dram_parameter("xsh", [NO, C], f32, isOutput=False)
    w_in = nc.declare_dram_parameter("w_in", [C, C], f32, isOutput=False)
    w1 = nc.declare_dram_parameter("w1", [C, C], f32, isOutput=False)
    w2a = nc.declare_dram_parameter("w2a", [C, C], f32, isOutput=False)
    w2b = nc.declare_dram_parameter("w2b", [C, C], f32, isOutput=False)
    w3h = nc.declare_dram_parameter("w3h", [C, C], f32, isOutput=False)
    b_in = nc.declare_dram_parameter("b_in", [C, 1], f32, isOutput=False)
    b2d = nc.declare_dram_parameter("b2", [C, 1], f32, isOutput=False)
    b3d = nc.declare_dram_parameter("b3", [C, 1], f32, isOutput=False)
    b1e_d = nc.declare_dram_parameter("b1e", [C, EO], f32, isOutput=False)
    dvrep_d = nc.declare_dram_parameter("dvrep", [C, NO], f32, isOutput=False)
    idxA_d = nc.declare_dram_parameter("idxA", [128, LA // 16], i16, isOutput=False)
    pa_d = nc.declare_dram_parameter("PA", [128, LA], f16, isOutput=False)
    idxB_d = nc.declare_dram_parameter("idxB", [128, LB // 16], i16, isOutput=False)
    pb_d = nc.declare_dram_parameter("PB", [128, LB], f16, isOutput=False)
    xout = nc.declare_dram_parameter("xout", [NO, C], f32, isOutput=True)

    # ---- internal DRAM ----
    agx_in = [nc.dram_tensor(f"agx_in{l}", [NO, C], SEG_DT) for l in range(N_LAYERS)]
    x_full = [
        nc.dram_tensor(f"x_full{l}", [N, C], SEG_DT, addr_space=X_FULL_ADDR_SPACE)
        for l in range(N_LAYERS)
    ]
    agxe_in = [nc.dram_tensor(f"agxe_in{l}", [EO, C], SEG_DT) for l in range(N_LAYERS)]
    xe_full = [
        nc.dram_tensor(f"xe_full{l}", [M, C], SEG_DT, addr_space=X_FULL_ADDR_SPACE)
        for l in range(N_LAYERS)
    ]

    rg = [list(range(R))]
    qrr = [0]  # round-robin SWDGE queue assignment for gathers

    def next_q():
        q = qrr[0]
        qrr[0] = (q + 1) % 4
        return q

    with tile.TileContext(nc) as tc:
        with (
            tc.tile_pool(name="const", bufs=1) as cp,
            tc.tile_pool(name="work", bufs=3) as wp,
            tc.tile_pool(name="ptiles", bufs=3) as pp,
            tc.tile_pool(name="psA", bufs=2, space="PSUM") as psA,
            tc.tile_pool(name="psB", bufs=2, space="PSUM") as psB,
            tc.tile_pool(name="psC", bufs=2, space="PSUM") as psC,
            tc.tile_pool(name="psD", bufs=2, space="PSUM") as psD,
        ):
            # ---------- persistent tiles ----------
            W_in = cp.tile([C, C], f32)
            W1 = cp.tile([C, C], f32)
            W2a = cp.tile([C, C], f32)
            W2b = cp.tile([C, C], f32)
            W3h = cp.tile([C, C], f32)
            Bin = cp.tile([C, 1], f32)
            B2 = cp.tile([C, 1], f32)
            B3 = cp.tile([C, 1], f32)
            B1E = cp.tile([C, EO], f32)
            DV = cp.tile([C, NO], f32)
            IDXA = cp.tile([128, LA // 16], i16)
            IDXB = cp.tile([128, LB // 16], i16)
            XFM = cp.tile([C, NO], f32)
            X0FM = cp.tile([C, NO], f32)
            IOTA = cp.tile([128, 128], f32)
            PIDX = cp.tile([128, 1], f32)
            IDENT = cp.tile([128, 128], f32)

            for t, d in [
                (W_in, w_in), (W1, w1), (W2a, w2a), (W2b, w2b), (W3h, w3h),
                (Bin, b_in), (B2, b2d), (B3, b3d), (B1E, b1e_d), (DV, dvrep_d),
                (IDXA, idxA_d), (IDXB, idxB_d),
            ]:
                nc.sync.dma_start(t[:], d[:])

            nc.gpsimd.iota(IOTA[:], [[1, 128]], channel_multiplier=0,
                           allow_small_or_imprecise_dtypes=True)
            nc.gpsimd.iota(PIDX[:], [[1, 1]], channel_multiplier=1,
                           allow_small_or_imprecise_dtypes=True)
            nc.vector.tensor_scalar(IDENT[:], IOTA[:], PIDX[:], None,
                                    mybir.AluOpType.is_equal)

            def seg_superchunk(gt, nch_list, p_dram, slot0, psum_out):
                """psum_out[C, 128] = sum over chunks of gt[:,c,:].T @ P_c,
                with the one-hot P tiles streamed from DRAM (host-built)."""
                ntot = sum(nch_list)
                if ntot == 0:
                    P = pp.tile([128, 128], SEG_DT, tag="P")
                    nc.vector.memset(P[:], 0.0)
                    nc.tensor.matmul(psum_out[:], P[:], P[:], start=True, stop=True)
                    return
                P = pp.tile([128, ntot * 128], SEG_DT, tag="P")
                nc.sync.dma_start(P[:], p_dram[:, slot0 : slot0 + ntot * 128])
                for c in range(ntot):
                    nc.tensor.matmul(
                        psum_out[:], gt[:, c, :], P[:, c * 128 : (c + 1) * 128],
                        start=(c == 0), stop=(c == ntot - 1),
                    )

            STG = DEBUG_STAGE
            # ---------- prologue: x = relu(x @ W_in + b_in), build XFM/X0FM ----------
            for s in range(SCB):
                n0 = s * 128
                ns = min(128, NO - n0)
                xin = wp.tile([128, C], f32, tag="xrm")
                nc.sync.dma_start(xin[:ns, :], xsh[n0 : n0 + ns, :])
                ptr = psD.tile([128, 128], f32, tag="tr")
                nc.tensor.transpose(ptr[:], xin[:], IDENT[:])
                xT = wp.tile([C, 128], f32, tag="xT")
                nc.vector.tensor_copy(xT[:, :ns], ptr[:, :ns])
                pmm = psB.tile([C, 128], f32, tag="mm")
                nc.tensor.matmul(pmm[:, :ns], W_in[:], xT[:, :ns])
                nc.scalar.activation(XFM[:, n0 : n0 + ns], pmm[:, :ns],
                                     mybir.ActivationFunctionType.Relu,
                                     bias=Bin[:, :1])
                nc.vector.tensor_copy(X0FM[:, n0 : n0 + ns], XFM[:, n0 : n0 + ns])
                # back to row-major for the gather source
                ptr2 = psD.tile([128, 128], f32, tag="tr")
                nc.tensor.transpose(ptr2[:], XFM[:, n0 : n0 + 128] if ns == 128
                                    else XFM[:, NO - 128 : NO], IDENT[:])
                xrm2 = wp.tile([128, C], SEG_DT, tag="xrm2")
                if ns == 128:
                    nc.vector.tensor_copy(xrm2[:], ptr2[:])
                    nc.sync.dma_start(agx_in[0][n0 : n0 + 128, :], xrm2[:])
                else:
                    # transposed the last 128 cols; valid rows are the tail
                    nc.vector.tensor_copy(xrm2[:], ptr2[:])
                    nc.sync.dma_start(agx_in[0][n0 : n0 + ns, :],
                                      xrm2[128 - ns :, :])
            if STG >= 1:
                nc.gpsimd.collective_compute(
                    "AllGather", mybir.AluOpType.bypass, replica_groups=rg,
                    ins=[agx_in[0][:]], outs=[x_full[0][:]],
                )

            # ---------- conv layers ----------
            for l in range(N_LAYERS):
                if STG < 2 + 4 * l:
                    break
                xf = x_full[l]
                # ---- phase A: Xe = (segsum_E x[V]) @ W1 + B1E ----
                colA = 0
                slotA = 0
                for s in range(SCA):
                    e0l = s * 128
                    ne = min(128, EO - e0l)
                    na, nb = nchA[s], nchB[s]
                    gt = wp.tile([128, max(na + nb, 1), C], SEG_DT, tag="gath")

                    def split_gather(src_ap, col0, nch_tot, slot0, nsplit=2):
                        # split one logical gather into nsplit calls on
                        # different SWDGE queues (parallel descriptor gen)
                        done = 0
                        while done < nch_tot:
                            step = max(1, (nch_tot + nsplit - 1) // nsplit)
                            step = min(step, nch_tot - done)
                            sl = slot0 + done * 128
                            nc.gpsimd.dma_gather(
                                out_ap=gt[:, col0 + done : col0 + done + step, :],
                                in_ap=src_ap,
                                idxs_ap=IDXA[:, sl // 16 : (sl + step * 128) // 16],
                                num_idxs=step * 128, num_idxs_reg=step * 128,
                                elem_size=C, single_packet=False,
                                queue_num=next_q(),
                            )
                            done += step

                    if na > 0:
                        split_gather(xf[:], 0, na, slotA)
                        slotA += na * 128
                    if nb > 0:
                        split_gather(xf[SPLIT:, :], na, nb, slotA)
                        slotA += nb * 128
                    if PA_LEVEL < 2:
                        nc.sync.dma_start(agxe_in[l][e0l : e0l + ne, :],
                                          gt[:ne, 0, :])
                        continue
                    pg = psA.tile([C, 128], f32, tag="seg")
                    seg_superchunk(gt, [na, nb], pa_d, colA * 128, pg)
                    colA += na + nb
                    gsb = wp.tile([C, 128], f32, tag="gsb")
                    nc.vector.tensor_copy(gsb[:], pg[:])
                    if PA_LEVEL < 3:
                        nc.sync.dma_start(agxe_in[l][e0l : e0l + ne, :],
                                          gsb[:ne, :])
                        continue
                    pxe = psB.tile([C, 128], f32, tag="mm")
                    nc.tensor.matmul(pxe[:, :ne], W1[:], gsb[:, :ne])
                    xesb = wp.tile([C, 128], f32, tag="xesb")
                    nc.vector.tensor_tensor(
                        xesb[:, :ne], pxe[:, :ne], B1E[:, e0l : e0l + ne],
                        mybir.AluOpType.add,
                    )
                    if PA_LEVEL < 4:
                        nc.sync.dma_start(agxe_in[l][e0l : e0l + ne, :],
                                          xesb[:ne, :])
                        continue
                    ptr = psD.tile([128, 128], f32, tag="tr")
                    nc.tensor.transpose(ptr[:ne, :], xesb[:, :ne], IDENT[:])
                    xerm = wp.tile([128, C], SEG_DT, tag="xerm")
                    nc.vector.tensor_copy(xerm[:ne, :], ptr[:ne, :])
                    nc.sync.dma_start(agxe_in[l][e0l : e0l + ne, :], xerm[:ne, :])
                if STG < 3 + 4 * l:
                    break
                nc.gpsimd.collective_compute(
                    "AllGather", mybir.AluOpType.bypass, replica_groups=rg,
                    ins=[agxe_in[l][:]], outs=[xe_full[l][:]],
                )

                # ---- phase B ----
                if STG < 4 + 4 * l:
                    break
                colB = 0
                slotB = 0
                for s in range(SCB):
                    n0 = s * 128
                    ns = min(128, NO - n0)
                    nch = nch2[s]
                    gt = wp.tile([128, max(nch, 1), C], SEG_DT, tag="gath")
                    done = 0
                    while done < nch:
                        step = min(max(1, (nch + 3) // 4), nch - done)
                        sl = slotB + done * 128
                        nc.gpsimd.dma_gather(
                            out_ap=gt[:, done : done + step, :], in_ap=xe_full[l][:],
                            idxs_ap=IDXB[:, sl // 16 : (sl + step * 128) // 16],
                            num_idxs=step * 128, num_idxs_reg=step * 128,
                            elem_size=C, single_packet=False, queue_num=next_q(),
                        )
                        done += step
                    slotB += nch * 128
                    py = psA.tile([C, 128], f32, tag="seg")
                    seg_superchunk(gt, [nch], pb_d, colB * 128, py)
                    colB += nch
                    ysb = wp.tile([C, 128], f32, tag="gsb")
                    nc.vector.tensor_copy(ysb[:], py[:])
                    # xdeg = x (*) deg_V
                    xdeg = wp.tile([C, 128], f32, tag="xdeg")
                    nc.vector.tensor_tensor(
                        xdeg[:, :ns], XFM[:, n0 : n0 + ns], DV[:, n0 : n0 + ns],
                        mybir.AluOpType.mult,
                    )
                    pab = psB.tile([C, 128], f32, tag="mm")
                    nc.tensor.matmul(pab[:, :ns], W2a[:], xdeg[:, :ns],
                                     start=True, stop=False)
                    nc.tensor.matmul(pab[:, :ns], W2b[:], ysb[:, :ns],
                                     start=False, stop=True)
                    # xv = pab + deg*b2; xmid = xv + x0
                    db2 = wp.tile([C, 128], f32, tag="db2")
                    nc.vector.tensor_scalar(db2[:, :ns], DV[:, n0 : n0 + ns],
                                            B2[:, :1], None, mybir.AluOpType.mult)
                    xv = wp.tile([C, 128], f32, tag="xv")
                    nc.vector.tensor_tensor(xv[:, :ns], pab[:, :ns], db2[:, :ns],
                                            mybir.AluOpType.add)
                    xmid = wp.tile([C, 128], f32, tag="xmid")
                    nc.vector.tensor_tensor(xmid[:, :ns], xv[:, :ns],
                                            X0FM[:, n0 : n0 + ns],
                                            mybir.AluOpType.add)
                    pc = psC.tile([C, 128], f32, tag="out")
                    nc.tensor.matmul(pc[:, :ns], W3h[:], xmid[:, :ns])
                    nc.scalar.activation(XFM[:, n0 : n0 + ns], pc[:, :ns],
                                         mybir.ActivationFunctionType.Relu,
                                         bias=B3[:, :1])
                    # row-major out (next gather source / final output)
                    ptr2 = psD.tile([128, 128], f32, tag="tr")
                    src = XFM[:, n0 : n0 + 128] if ns == 128 else XFM[:, NO - 128 : NO]
                    nc.tensor.transpose(ptr2[:], src, IDENT[:])
                    xrm = wp.tile([128, C], f32 if l == N_LAYERS - 1 else SEG_DT, tag="xrm_f" if l == N_LAYERS - 1 else "xrm2")
                    nc.vector.tensor_copy(xrm[:], ptr2[:])
                    dst = xout if l == N_LAYERS - 1 else agx_in[l + 1]
                    if ns == 128:
                        nc.sync.dma_start(dst[n0 : n0 + 128, :], xrm[:])
                    else:
                        nc.sync.dma_start(dst[n0 : n0 + ns, :], xrm[128 - ns :, :])
                if STG < 5 + 4 * l:
                    break
                if l < N_LAYERS - 1:
                    nc.gpsimd.collective_compute(
                        "AllGather", mybir.AluOpType.bypass, replica_groups=rg,
                        ins=[agx_in[l + 1][:]], outs=[x_full[l + 1][:]],
                    )
    nc.compile()
    return nc


def _get_program(V, E):
    key = (hash(V.tobytes()), hash(E.tobytes()))
    if key not in _cache:
        meta, per_core = _prepare(V, E)
        nc = _build(meta)
        _cache[key] = (nc, per_core)
    return _cache[key]


def run(trace=False, trace_kwargs=None, **inputs):
    x = np.ascontiguousarray(np.asarray(inputs["x"], dtype=np.float32))
    V = np.asarray(inputs["V"]).astype(np.int64)
    E = np.asarray(inputs["E"]).astype(np.int64)
    W_in = np.ascontiguousarray(np.asarray(inputs["W_in"], np.float32))
    b_in = np.asarray(inputs["b_in"], np.float32).reshape(C, 1)
    W1 = np.ascontiguousarray(np.asarray(inputs["W1"], np.float32))
    b1 = np.asarray(inputs["b1"], np.float32).reshape(C)
    W2 = np.asarray(inputs["W2"], np.float32)
    b2 = np.asarray(inputs["b2"], np.float32).reshape(C, 1)
    W3 = np.asarray(inputs["W3"], np.float32)
    b3 = np.asarray(inputs["b3"], np.float32).reshape(C, 1)
    W2a = np.ascontiguousarray(W2[:C])
    W2b = np.ascontiguousarray(W2[C:])
    W3h = np.ascontiguousarray((1.0 - ALPHA) * W3)
    # note: (1-a)*Xv + a*x0 = (1-a)*(Xv + x0) since a = 0.5

    nc, per_core = _get_program(V, E)

    in_maps = []
    for r in range(R):
        pc = per_core[r]
        b1e = np.ascontiguousarray(np.outer(b1, pc["degE"]).astype(np.float32))
        dvrep = np.ascontiguousarray(
            np.broadcast_to(pc["degV"], (C, NO)).astype(np.float32))
        in_maps.append({
            "xsh": x[r * NO : (r + 1) * NO],
            "w_in": W_in, "w1": W1, "w2a": W2a, "w2b": W2b, "w3h": W3h,
            "b_in": b_in, "b2": b2, "b3": b3,
            "b1e": b1e, "dvrep": dvrep,
            "idxA": pc["idxA"], "PA": pc["PA"],
            "idxB": pc["idxB"], "PB": pc["PB"],
        })
    res = run_bass_kernel_spmd(nc, in_maps, list(range(R)), trace=trace,
                               **(trace_kwargs or {}))
    out = np.concatenate([res.results[r]["xout"] for r in range(R)], axis=0)
    return out, res


def kernel(**inputs):
    out, _ = run(**inputs)
    return out

